# revision 2
# baseline (speedup 1.0000x reference)
"""4-layer GCN (N=100k, E=3.2M) on 8 TRN2 NeuronCores.

Strategy (graph-parallel, dst-sharded):
  - Fold symmetric normalization into row scalings:
        out_l = dis * (A_hat_agg) + b ;  with self-loops kept as ordinary edges
    where dis = deg^-1/2 and the aggregated quantity is y = dis * (h @ W).
    Then out[d] = dis[d] * sum_{(s,d) in E'} y[s] + b  exactly matches GCNConv.
  - Destination nodes are assigned to (core, window, slot) with degree
    balancing so every 128-dst window has nearly equal edge count.
  - Per layer: dense transform y = hs @ W on PE; per-window batched
    indirect-DMA gathers of y[src] rows (NB*128 rows per instruction);
    one-hot H built on DVE (is_equal vs iota); segment-sum via PE matmul
    accumulation into PSUM; epilogue scale+bias+relu; AllGather of the
    (transposed, pre-scaled) activations between layers.
"""

import sys

if "/opt/trn_rl_repo" not in sys.path:
    sys.path.insert(0, "/opt/trn_rl_repo")

import numpy as np

import concourse.bass as bass
import concourse.mybir as mybir
import concourse.tile as tile
from concourse import bacc
from concourse.bass import IndirectOffsetOnAxis
from concourse.bass_utils import run_bass_kernel_spmd

F32 = mybir.dt.float32
I32 = mybir.dt.int32
P = 128

# ---------------------------------------------------------------- host side


def _assign_windows(deg, n_cores, ns):
    """Assign each node to (core, window, slot) balancing per-window edge load.

    Windows per core: ns//P full windows plus one partial (ns % P slots).
    Returns node_order (node ids in global (core, window, slot) order) and
    pos_of (inverse permutation).
    """
    n = deg.shape[0]
    nw = (ns + P - 1) // P
    last_cap = ns - (nw - 1) * P
    n_win = n_cores * nw
    caps = np.full(n_win, P, np.int64)
    caps[nw - 1 :: nw] = last_cap  # last window of each core is partial

    order = np.argsort(-deg, kind="stable")
    # snake-deal nodes (sorted by degree desc) across windows; windows with
    # smaller capacity drop out of the tail rounds automatically
    slots_of = [[] for _ in range(n_win)]
    loads = np.zeros(n_win, np.int64)
    win_ids = np.arange(n_win)
    pos = 0
    rnd = 0
    while pos < n:
        take = win_ids if rnd % 2 == 0 else win_ids[::-1]
        for wgid in take:
            if pos >= n:
                break
            if len(slots_of[wgid]) < caps[wgid]:
                slots_of[wgid].append(order[pos])
                loads[wgid] += deg[order[pos]]
                pos += 1
        rnd += 1
    node_order = np.concatenate([np.array(s, np.int64) for s in slots_of])
    pos_of = np.empty(n, np.int64)
    pos_of[node_order] = np.arange(n)
    return node_order, pos_of, nw


def preprocess(x, edge_index, n_cores=8):
    """Build per-core packed inputs. Returns (cfg, in_maps_common, per_core)."""
    n = x.shape[0]
    assert n % n_cores == 0
    ns = n // n_cores

    src = np.asarray(edge_index[0], np.int64)
    dst = np.asarray(edge_index[1], np.int64)
    loops = np.arange(n, dtype=np.int64)
    src = np.concatenate([src, loops])
    dst = np.concatenate([dst, loops])

    deg = np.bincount(dst, minlength=n)  # includes self-loop
    node_order, pos_of, nw = _assign_windows(deg, n_cores, ns)

    # CSR over dst (for device-side deg computation): rowptr per node
    order_e = np.argsort(dst, kind="stable")
    dst_s = dst[order_e]
    src_s = src[order_e]
    rowptr = np.searchsorted(dst_s, np.arange(n + 1))

    # window-global id and slot of each node
    gpos = pos_of  # global position: core*ns + w*P + slot ... but last window
    # of each core is partial; positions within a core are contiguous 0..ns-1
    core_of_pos = gpos // ns
    off_in_core = gpos % ns
    w_of_pos = off_in_core // P
    slot_of_pos = off_in_core % P

    # per-edge: destination node -> (core, window, slot); src -> y row = pos_of
    e_core = core_of_pos[dst_s]
    e_w = w_of_pos[dst_s]
    e_slot = slot_of_pos[dst_s]
    e_srcpos = gpos[src_s].astype(np.int64)

    # global window index, stable-sort edges by it
    e_gw = e_core * nw + e_w
    eorder = np.argsort(e_gw, kind="stable")
    e_gw = e_gw[eorder]
    e_slot = e_slot[eorder]
    e_srcpos = e_srcpos[eorder]

    n_win = n_cores * nw
    win_cnt = np.bincount(e_gw, minlength=n_win)
    nb = int((win_cnt.max() + P - 1) // P)
    win_start = np.concatenate([[0], np.cumsum(win_cnt)])[:-1]
    # position of each edge within its window
    e_k = np.arange(e_gw.shape[0]) - win_start[e_gw]
    e_lane = (e_k % P).astype(np.int64)
    e_batch = (e_k // P).astype(np.int64)

    # packed arrays [n_cores][P, nw*nb]
    src_pack = np.zeros((n_cores, P, nw * nb), np.int32)
    slot_pack = np.full((n_cores, P, nw * nb), 999.0, np.float32)
    col = e_w[0] * 0  # dummy
    e_wl = e_gw % nw
    col = e_wl * nb + e_batch
    src_pack[e_gw // nw, e_lane, col] = e_srcpos.astype(np.int32)
    slot_pack[e_gw // nw, e_lane, col] = e_slot.astype(np.float32)

    # rowptr packs [n_cores][P, nw]
    rp0 = np.zeros((n_cores, P, nw), np.int32)
    rp1 = np.ones((n_cores, P, nw), np.int32)  # empty slots -> deg 1
    no = node_order  # global (core, w, slot) order, contiguous per core
    c_of = core_of_pos[no]
    w_of = w_of_pos[no]
    s_of = slot_of_pos[no]
    rp0[c_of, s_of, w_of] = rowptr[no].astype(np.int32)
    rp1[c_of, s_of, w_of] = (rowptr[no] + deg[no]).astype(np.int32)

    # x shards in assignment order
    x_sh = np.ascontiguousarray(x[node_order].reshape(n_cores, ns, x.shape[1])).astype(
        np.float32
    )

    cfg = dict(n=n, ns=ns, nw=nw, nb=nb, n_cores=n_cores)
    per_core = dict(src_pack=src_pack, slot_pack=slot_pack, rp0=rp0, rp1=rp1, x_sh=x_sh)
    return cfg, per_core, node_order


# ---------------------------------------------------------------- device side


def build(cfg, dims):
    """Build the SPMD bass program. dims = [(6,32),(32,64),(64,128),(128,2)]."""
    n, ns, nw, nb, n_cores = cfg["n"], cfg["ns"], cfg["nw"], cfg["nb"], cfg["n_cores"]
    fin0 = dims[0][0]
    f3 = dims[2][1]  # 128
    nwnb = nw * nb
    rg = [list(range(n_cores))]

    nc = bacc.Bacc(None, target_bir_lowering=False)

    # ---- I/O
    x_in = nc.dram_tensor("x_sh", [ns, fin0], F32, kind="ExternalInput")
    srcp_in = nc.dram_tensor("src_pack", [P, nwnb], I32, kind="ExternalInput")
    slotp_in = nc.dram_tensor("slot_pack", [P, nwnb], F32, kind="ExternalInput")
    rp0_in = nc.dram_tensor("rp0", [P, nw], I32, kind="ExternalInput")
    rp1_in = nc.dram_tensor("rp1", [P, nw], I32, kind="ExternalInput")
    w_in, bt_in = [], []
    for li, (fi, fo) in enumerate(dims):
        w_in.append(nc.dram_tensor(f"W{li + 1}", [fi, fo], F32, kind="ExternalInput"))
        bt_in.append(nc.dram_tensor(f"bt{li + 1}", [P, fo], F32, kind="ExternalInput"))
    iota_in = nc.dram_tensor("iota", [P, P], F32, kind="ExternalInput")
    id_in = nc.dram_tensor("ident", [P, P], F32, kind="ExternalInput")
    out_t = nc.dram_tensor("out", [ns, dims[3][1]], F32, kind="ExternalOutput")

    # ---- internal DRAM
    xsT = nc.dram_tensor("xsT", [fin0, ns], F32)
    ag = [nc.dram_tensor("ag1", [n_cores * fin0, ns], F32)]
    hsT = []
    for li in range(1, 3):
        fo = dims[li - 1][1]
        hsT.append(nc.dram_tensor(f"hs{li}T", [fo, ns], F32))
        ag.append(nc.dram_tensor(f"ag{li + 1}", [n_cores * fo, ns], F32))
    y = [
        nc.dram_tensor(f"y{li + 1}", [n, dims[li][1]], F32) for li in range(3)
    ]
    y4sh = nc.dram_tensor("y4sh", [ns, dims[3][1]], F32)
    ag4 = nc.dram_tensor("ag4", [n, dims[3][1]], F32)

    last_ws = ns - (nw - 1) * P

    with tile.TileContext(nc) as tc:
        with (
            tc.tile_pool(name="const", bufs=1) as cpool,
            tc.tile_pool(name="gpool", bufs=2) as gpool,
            tc.tile_pool(name="hpool", bufs=2) as hpool,
            tc.tile_pool(name="spool", bufs=3) as spool,
            tc.tile_pool(name="tfpool", bufs=3) as tfpool,
            tc.tile_pool(name="acc", bufs=2, space="PSUM") as accp,
            tc.tile_pool(name="tp", bufs=3, space="PSUM") as tpp,
        ):
            # ---------- constants / resident tiles
            iota_t = cpool.tile([P, P], F32, tag="iota")
            nc.sync.dma_start(iota_t[:], iota_in[:, :])
            id_t = cpool.tile([P, P], F32, tag="ident")
            nc.sync.dma_start(id_t[:], id_in[:, :])
            w_t, bt_t = [], []
            for li, (fi, fo) in enumerate(dims):
                wt = cpool.tile([fi, fo], F32, tag=f"w{li}")
                nc.sync.dma_start(wt[:], w_in[li][:, :])
                w_t.append(wt)
                bt = cpool.tile([P, fo], F32, tag=f"bt{li}")
                nc.sync.dma_start(bt[:], bt_in[li][:, :])
                bt_t.append(bt)
            src_t = cpool.tile([P, nwnb], I32, tag="srcp")
            nc.sync.dma_start(src_t[:], srcp_in[:, :])
            slot_t = cpool.tile([P, nwnb], F32, tag="slotp")
            nc.sync.dma_start(slot_t[:], slotp_in[:, :])

            # ---------- degree -> dis = 1/sqrt(deg)
            rp0_t = cpool.tile([P, nw], I32, tag="rp0")
            nc.sync.dma_start(rp0_t[:], rp0_in[:, :])
            rp1_t = cpool.tile([P, nw], I32, tag="rp1")
            nc.sync.dma_start(rp1_t[:], rp1_in[:, :])
            cnt_t = cpool.tile([P, nw], I32, tag="cnt")
            nc.vector.tensor_tensor(
                out=cnt_t[:], in0=rp1_t[:], in1=rp0_t[:], op=mybir.AluOpType.subtract
            )
            deg_t = cpool.tile([P, nw], F32, tag="deg")
            nc.vector.tensor_copy(out=deg_t[:], in_=cnt_t[:])
            rec_t = cpool.tile([P, nw], F32, tag="rec")
            nc.vector.reciprocal(out=rec_t[:], in_=deg_t[:])
            dis_t = cpool.tile([P, nw], F32, tag="dis")
            nc.scalar.sqrt(out=dis_t[:], in_=rec_t[:])

            # hs3T stays resident in SBUF
            hs3T_t = cpool.tile([f3, ns], F32, tag="hs3T")

            # ---------- phase 0: xs = dis*x, transposed to xsT
            for w in range(nw):
                ws = P if w < nw - 1 else last_ws
                t0 = w * P
                xt = spool.tile([P, fin0], F32, tag="xt")
                nc.sync.dma_start(xt[:ws], x_in[t0 : t0 + ws, :])
                xs = spool.tile([P, fin0], F32, tag="xs")
                nc.scalar.activation(
                    out=xs[:ws],
                    in_=xt[:ws],
                    func=mybir.ActivationFunctionType.Copy,
                    scale=dis_t[:ws, w : w + 1],
                )
                xT_ps = tpp.tile([fin0, P], F32, tag="tp")
                nc.tensor.transpose(out=xT_ps[:, :ws], in_=xs[:ws], identity=id_t[:ws, :ws])
                xT_sb = spool.tile([fin0, P], F32, tag="xTsb")
                nc.vector.tensor_copy(out=xT_sb[:, :ws], in_=xT_ps[:, :ws])
                nc.sync.dma_start(xsT[:, t0 : t0 + ws], xT_sb[:, :ws])

            nc.gpsimd.collective_compute(
                "AllGather",
                mybir.AluOpType.bypass,
                replica_groups=rg,
                ins=[xsT.ap().opt()],
                outs=[ag[0].ap().opt()],
            )

            # ---------- layers
            for li in range(4):
                fi, fo = dims[li]
                if li < 3:
                    # transform: y_li[pos] = hsT_block.T @ W  (row-major y)
                    agl = ag[li]
                    for cb in range(n_cores):
                        for t in range(nw):
                            ws = P if t < nw - 1 else last_ws
                            t0 = t * P
                            lt = tfpool.tile([fi, P], F32, tag="lt")
                            nc.sync.dma_start(
                                lt[:, :ws], agl[cb * fi : (cb + 1) * fi, t0 : t0 + ws]
                            )
                            yp = accp.tile([P, fo], F32, tag="acc")
                            nc.tensor.matmul(
                                out=yp[:ws],
                                lhsT=lt[:, :ws],
                                rhs=w_t[li][:],
                                start=True,
                                stop=True,
                            )
                            ysb = tfpool.tile([P, fo], F32, tag="ysb")
                            nc.vector.tensor_copy(out=ysb[:ws], in_=yp[:ws])
                            nc.sync.dma_start(
                                y[li][cb * ns + t0 : cb * ns + t0 + ws, :], ysb[:ws]
                            )
                    ysrc = y[li]
                else:
                    # layer 4 transform from resident hs3T, then AllGather
                    for t in range(nw):
                        ws = P if t < nw - 1 else last_ws
                        t0 = t * P
                        yp = accp.tile([P, fo], F32, tag="acc")
                        nc.tensor.matmul(
                            out=yp[:ws],
                            lhsT=hs3T_t[:, t0 : t0 + ws],
                            rhs=w_t[3][:],
                            start=True,
                            stop=True,
                        )
                        ysb = tfpool.tile([P, fo], F32, tag="ysb")
                        nc.vector.tensor_copy(out=ysb[:ws], in_=yp[:ws])
                        nc.sync.dma_start(y4sh[t0 : t0 + ws, :], ysb[:ws])
                    nc.gpsimd.collective_compute(
                        "AllGather",
                        mybir.AluOpType.bypass,
                        replica_groups=rg,
                        ins=[y4sh.ap().opt()],
                        outs=[ag4.ap().opt()],
                    )
                    ysrc = ag4

                # aggregation over windows
                for w in range(nw):
                    ws = P if w < nw - 1 else last_ws
                    t0 = w * P
                    g = gpool.tile([P, nb * fo], F32, tag="g")
                    nc.gpsimd.indirect_dma_start(
                        out=g[:],
                        out_offset=None,
                        in_=ysrc[:, :],
                        in_offset=IndirectOffsetOnAxis(
                            ap=src_t[:, w * nb : (w + 1) * nb], axis=0
                        ),
                    )
                    h = hpool.tile([P, nb * P], F32, tag="h")
                    nc.vector.tensor_tensor(
                        out=h[:].rearrange("p (b s) -> p b s", b=nb),
                        in0=slot_t[:, w * nb : (w + 1) * nb].to_broadcast([P, nb, P]),
                        in1=iota_t[:].rearrange("p (b s) -> p b s", b=1).to_broadcast(
                            [P, nb, P]
                        ),
                        op=mybir.AluOpType.is_equal,
                    )
                    zT = accp.tile([fo, P], F32, tag="acc")
                    for b in range(nb):
                        nc.tensor.matmul(
                            out=zT[:],
                            lhsT=g[:, b * fo : (b + 1) * fo],
                            rhs=h[:, b * P : (b + 1) * P],
                            start=(b == 0),
                            stop=(b == nb - 1),
                        )
                    zT_sb = spool.tile([fo, P], F32, tag="zTsb")
                    nc.vector.tensor_copy(out=zT_sb[:], in_=zT[:])
                    z_ps = tpp.tile([P, fo], F32, tag="tp")
                    nc.tensor.transpose(
                        out=z_ps[:ws], in_=zT_sb[:, :ws], identity=id_t[:fo, :fo]
                    )
                    u = spool.tile([P, fo], F32, tag="u")
                    nc.scalar.activation(
                        out=u[:ws],
                        in_=z_ps[:ws],
                        func=mybir.ActivationFunctionType.Copy,
                        scale=dis_t[:ws, w : w + 1],
                    )
                    v = spool.tile([P, fo], F32, tag="v")
                    nc.vector.tensor_tensor(
                        out=v[:ws], in0=u[:ws], in1=bt_t[li][:ws], op=mybir.AluOpType.add
                    )
                    if li < 3:
                        hs = spool.tile([P, fo], F32, tag="hs")
                        nc.scalar.activation(
                            out=hs[:ws],
                            in_=v[:ws],
                            func=mybir.ActivationFunctionType.Relu,
                            scale=dis_t[:ws, w : w + 1],
                        )
                        hsT_ps = tpp.tile([fo, P], F32, tag="tp")
                        nc.tensor.transpose(
                            out=hsT_ps[:, :ws], in_=hs[:ws], identity=id_t[:ws, :ws]
                        )
                        if li == 2:
                            nc.vector.tensor_copy(
                                out=hs3T_t[:, t0 : t0 + ws], in_=hsT_ps[:, :ws]
                            )
                        else:
                            hsT_sb = spool.tile([fo, P], F32, tag="hsTsb")
                            nc.vector.tensor_copy(out=hsT_sb[:, :ws], in_=hsT_ps[:, :ws])
                            nc.sync.dma_start(
                                hsT[li][:, t0 : t0 + ws], hsT_sb[:, :ws]
                            )
                    else:
                        # log_softmax over the 2 classes
                        nm = spool.tile([P, 1], F32, tag="nm")
                        nc.vector.tensor_reduce(
                            out=nm[:ws],
                            in_=v[:ws],
                            op=mybir.AluOpType.max,
                            axis=mybir.AxisListType.X,
                            negate=True,
                        )
                        e = spool.tile([P, fo], F32, tag="e")
                        nc.scalar.activation(
                            out=e[:ws],
                            in_=v[:ws],
                            func=mybir.ActivationFunctionType.Exp,
                            bias=nm[:ws],
                        )
                        s = spool.tile([P, 1], F32, tag="s")
                        nc.vector.tensor_reduce(
                            out=s[:ws],
                            in_=e[:ws],
                            op=mybir.AluOpType.add,
                            axis=mybir.AxisListType.X,
                        )
                        ls = spool.tile([P, 1], F32, tag="ls")
                        nc.scalar.activation(
                            out=ls[:ws],
                            in_=s[:ws],
                            func=mybir.ActivationFunctionType.Ln,
                        )
                        r = spool.tile([P, fo], F32, tag="r")
                        nc.vector.tensor_scalar(
                            out=r[:ws],
                            in0=v[:ws],
                            scalar1=nm[:ws],
                            scalar2=ls[:ws],
                            op0=mybir.AluOpType.add,
                            op1=mybir.AluOpType.subtract,
                        )
                        nc.sync.dma_start(out_t[t0 : t0 + ws, :], r[:ws])

                if li < 2:
                    nc.gpsimd.collective_compute(
                        "AllGather",
                        mybir.AluOpType.bypass,
                        replica_groups=rg,
                        ins=[hsT[li].ap().opt()],
                        outs=[ag[li + 1].ap().opt()],
                    )

    nc.finalize()
    return nc


# ---------------------------------------------------------------- entry point


def kernel(x, edge_index, W1, b1, W2, b2, W3, b3, W4, b4, _trace=False):
    x = np.asarray(x, np.float32)
    n_cores = 8
    cfg, per_core, node_order = preprocess(x, np.asarray(edge_index), n_cores)
    dims = [
        (W1.shape[0], W1.shape[1]),
        (W2.shape[0], W2.shape[1]),
        (W3.shape[0], W3.shape[1]),
        (W4.shape[0], W4.shape[1]),
    ]
    nc = build(cfg, dims)

    ws_list = [np.asarray(w, np.float32) for w in (W1, W2, W3, W4)]
    bs_list = [np.asarray(b, np.float32) for b in (b1, b2, b3, b4)]
    common = dict(
        iota=np.tile(np.arange(P, dtype=np.float32), (P, 1)),
        ident=np.eye(P, dtype=np.float32),
    )
    for li in range(4):
        common[f"W{li + 1}"] = ws_list[li]
        common[f"bt{li + 1}"] = np.tile(bs_list[li], (P, 1))

    in_maps = []
    for c in range(n_cores):
        m = dict(common)
        m["x_sh"] = per_core["x_sh"][c]
        m["src_pack"] = per_core["src_pack"][c]
        m["slot_pack"] = per_core["slot_pack"][c]
        m["rp0"] = per_core["rp0"][c]
        m["rp1"] = per_core["rp1"][c]
        in_maps.append(m)

    res = run_bass_kernel_spmd(nc, in_maps, list(range(n_cores)), trace=_trace)
    outs = np.concatenate([res.results[c]["out"] for c in range(n_cores)], axis=0)
    full = np.empty((cfg["n"], dims[3][1]), np.float32)
    full[node_order] = outs
    if _trace:
        return full, res
    return full
